# revision 14
# baseline (speedup 1.0000x reference)
"""Cross-attention Bass/Tile kernel for Trainium2, data-parallel over batch.

Problem (per batch element b, 8 of them, one per NeuronCore):
    q = x1 @ Wq + bq                      [2048, 512]
    k = x2 @ Wk + bk                      [2048, 512]
    v = x2 @ Wv + bv                      [2048, 512]
    scores  = q @ k.T                     [2048, 2048]  (no 1/sqrt(d))
    weights = softmax(scores, axis=-1)
    fused   = layer_norm(weights @ v) * gamma + beta
    out     = fused @ Wo + bo             [2048, 128]

Tricks relative to a naive lowering:
  * layer_norm is invariant to a per-row positive scale, so the softmax
    denominator is never computed: weights = exp(scores - 21) feed the AV
    matmul unnormalized and LN absorbs the scale.
  * x1/x2 are transposed AND tiled on the HOST (k-on-partitions layout), so
    the kernel has zero input transposes on the PE and zero PSUM->SBUF
    eviction copies for them.  Weights/biases are likewise host-packed;
    gamma/beta are folded into Wo/bo on the host.
  * rsqrt for LN is computed as exp(-0.5*ln(var+eps)): Ln/Exp/Identity all
    live in the same activation table set, so the Act engine never reloads
    tables (Sqrt would force 2 reloads x 1283ns per slice).
  * The post-LN path runs in bf16 (lnT transposes at 1.0 cycles/row, final
    projection as 128-wide natural-layout matmuls), so the final output
    needs no PE transposes back.  Everything through the AV matmul stays
    float32r (fp32 bits, 1 cycle/row at free-dim >= 256).
  * The prologue is chunk-pipelined: each 512-row chunk of x2 is consumed by
    K^T/V matmuls as soon as its DMA lands, instead of waiting for all of
    x2 to arrive.

build(n_loop=k) wraps the body in tc.For_i for slope benchmarking; the
graded kernel uses k=1.
"""

import numpy as np
import ml_dtypes

import concourse.bass as bass
import concourse.mybir as mybir
import concourse.tile as tile
from concourse import bacc
from concourse.bass_utils import run_bass_kernel_spmd
from concourse.masks import make_identity

# Problem dims (hardcoded per the harness contract)
B, N, M, K, D, F = 8, 2048, 2048, 512, 512, 128
P = 128
KO = K // P      # 4 contraction chunks over D_IN
DC = D // P      # 4 chunks over D_OUT
MT = M // P      # 16 m-chunks
MS = M // 512    # 4 m-slices of 512 (prologue chunks)
NS = N // 512    # 4 n-slices of 512
NSUB = 512 // P  # 4 n-subtiles per slice

FP32 = mybir.dt.float32
FP32R = mybir.dt.float32r
FP16 = mybir.dt.float16
BF16 = mybir.dt.bfloat16
EXP_SHIFT = -21.0
AF = mybir.ActivationFunctionType
ALU = mybir.AluOpType


def build(n_loop=1, loop_scope="all"):
    nc = bacc.Bacc(None, target_bir_lowering=False)

    x1p = nc.dram_tensor("x1p", [NS, P, KO, 512], FP32, kind="ExternalInput")
    x2p = nc.dram_tensor("x2p", [MS, P, KO, 512], FP32, kind="ExternalInput")
    wqp = nc.dram_tensor("wqp", [P, KO, D], FP32, kind="ExternalInput")
    wkp = nc.dram_tensor("wkp", [P, KO, D], FP32, kind="ExternalInput")
    wvp = nc.dram_tensor("wvp", [P, KO, D], FP32, kind="ExternalInput")
    wop = nc.dram_tensor("wop", [P, DC, F], BF16, kind="ExternalInput")
    bqp = nc.dram_tensor("bqp", [P, DC], FP32, kind="ExternalInput")
    bkp = nc.dram_tensor("bkp", [P, DC], FP32, kind="ExternalInput")
    bvb = nc.dram_tensor("bvb", [P, D], FP32, kind="ExternalInput")
    bob = nc.dram_tensor("bob", [P, F], FP32, kind="ExternalInput")
    csb = nc.dram_tensor("csb", [P, F], FP32, kind="ExternalInput")
    out = nc.dram_tensor("out", [N, F], FP32, kind="ExternalOutput")

    with tile.TileContext(nc) as tc:
        with (
            tc.tile_pool(name="const", bufs=1) as constp,
            tc.tile_pool(name="wtmp", bufs=1) as wtmp,
            tc.tile_pool(name="big", bufs=1) as bigp,
            tc.tile_pool(name="expp", bufs=MT) as expp,
            tc.tile_pool(name="x1tp", bufs=NS) as x1tp,
            tc.tile_pool(name="qtp", bufs=2) as qtp,
            tc.tile_pool(name="lnp", bufs=3) as lnp,
            tc.tile_pool(name="lntp", bufs=2) as lntp,
            tc.tile_pool(name="outp", bufs=3) as outp,
            tc.tile_pool(name="stat", bufs=6) as stat,
            tc.tile_pool(name="ps", bufs=8, space="PSUM") as ps,
        ):
            # ---- constants (loaded once, outside any loop) ----
            ident = constp.tile([P, P], FP32)
            make_identity(nc, ident)
            identb = constp.tile([P, P], BF16)
            nc.vector.tensor_copy(out=identb, in_=ident)
            shift = constp.tile([P, 1], FP32)
            nc.vector.memset(shift, EXP_SHIFT)
            eps_t = constp.tile([P, 1], FP32)
            nc.vector.memset(eps_t, 1e-5)
            mhalf = constp.tile([P, 1], FP32)
            nc.vector.memset(mhalf, -0.5)
            mneg1 = constp.tile([P, 1], FP32)
            nc.vector.memset(mneg1, -1.0)
            bq_t = constp.tile([P, DC], FP32)
            nc.sync.dma_start(out=bq_t, in_=bqp[:, :])
            bk_t = constp.tile([P, DC], FP32)
            nc.sync.dma_start(out=bk_t, in_=bkp[:, :])
            bo_b = constp.tile([P, F], FP32)
            nc.sync.dma_start(out=bo_b, in_=bob[:, :])
            cs_b = constp.tile([P, F], FP32)
            nc.sync.dma_start(out=cs_b, in_=csb[:, :])
            bv_b = constp.tile([P, D], FP32)
            nc.sync.dma_start(out=bv_b, in_=bvb[:, :])
            wq_t = constp.tile([P, KO, D], FP32R)
            wo_t = constp.tile([P, DC, F], BF16)
            wk_t = wtmp.tile([P, KO, D], FP32R)
            wv_t = wtmp.tile([P, KO, D], FP32R)

            x2t = bigp.tile([P, KO, M], FP32R, tag="x2t", name="x2t")
            kt = bigp.tile([P, DC, M], FP16, tag="kt", name="kt")
            v_sb = bigp.tile([P, MT, D], BF16, tag="v", name="v_sb")
            x1ts = [x1tp.tile([P, KO, 512], FP32R, tag="x1t", name=f"x1t{i}")
                    for i in range(NS)]

            def body():
                # ---- input DMAs, ordered so the first consumers unblock
                # earliest: wk + x2 chunk0 feed the first K^T matmuls ----
                for ko in range(KO):
                    nc.gpsimd.dma_start(out=wk_t[:, ko, :], in_=wkp[:, ko, :])
                    nc.gpsimd.dma_start(out=x2t[:, ko, 0:512], in_=x2p[0][:, ko, :])
                for ko in range(KO):
                    nc.gpsimd.dma_start(out=wv_t[:, ko, :], in_=wvp[:, ko, :])
                    nc.gpsimd.dma_start(out=x2t[:, ko, 512:1024], in_=x2p[1][:, ko, :])
                nc.gpsimd.dma_start(out=wq_t, in_=wqp[:, :, :])
                nc.gpsimd.dma_start(out=x2t[:, :, 1024:1536], in_=x2p[2])
                nc.gpsimd.dma_start(out=x2t[:, :, 1536:2048], in_=x2p[3])
                nc.gpsimd.dma_start(out=wo_t, in_=wop[:, :, :])
                for i in range(NS):
                    nc.gpsimd.dma_start(out=x1ts[i], in_=x1p[i])

                # ---- prologue: K^T and V, chunk-pipelined over x2 chunks ----
                for ms in range(MS):
                    mw = slice(ms * 512, (ms + 1) * 512)
                    for dc in range(DC):
                        pk = ps.tile([P, 512], FP32, tag="ps")
                        for ko in range(KO):
                            nc.tensor.matmul(
                                pk, wk_t[:, ko, dc * P:(dc + 1) * P],
                                x2t[:, ko, mw],
                                start=(ko == 0), stop=(ko == KO - 1))
                        nc.scalar.activation(
                            out=kt[:, dc, mw], in_=pk,
                            func=AF.Identity, bias=bk_t[:, dc:dc + 1], scale=1.0)
                    for mtl in range(4):
                        mt = ms * 4 + mtl
                        pv = ps.tile([P, 512], FP32, tag="ps")
                        for ko in range(KO):
                            nc.tensor.matmul(
                                pv, x2t[:, ko, mt * P:(mt + 1) * P], wv_t[:, ko, :],
                                start=(ko == 0), stop=(ko == KO - 1))
                        nc.vector.tensor_add(out=v_sb[:, mt, :], in0=pv, in1=bv_b)

                # ---- per 512-wide n-slice ----
                # Layer_norm is applied AFTER the final projection:
                #   ln(f) @ Wo' = rstd*(f@Wo') - (mu*rstd)*colsum(Wo'), with
                # rstd/mu per output row (n on partitions in the natural
                # layout), so the raw bf16 AV output is transposed directly and
                # the whole LN stats chain runs off the PE critical path.
                qts = [None] * NS

                def emit_qt(ns):
                    qt = qtp.tile([P, DC, 512], FP16, tag="qt")
                    x1t = x1ts[ns]
                    for dc in range(DC):
                        pq = ps.tile([P, 512], FP32, tag="ps")
                        for ko in range(KO):
                            nc.tensor.matmul(
                                pq, wq_t[:, ko, dc * P:(dc + 1) * P], x1t[:, ko, :],
                                start=(ko == 0), stop=(ko == KO - 1))
                        nc.scalar.activation(
                            out=qt[:, dc, :], in_=pq,
                            func=AF.Identity, bias=bq_t[:, dc:dc + 1], scale=1.0)
                    qts[ns] = qt

                emit_qt(0)
                for ns in range(NS):
                    qt = qts[ns]

                    # transposed scores + fused exp eviction
                    expT = [expp.tile([P, 512], BF16, tag="expT", name=f"expT{i}")
                            for i in range(MT)]
                    for mt in range(MT):
                        psc = ps.tile([P, 512], FP32, tag="ps")
                        for dc in range(DC):
                            nc.tensor.matmul(
                                psc, kt[:, dc, mt * P:(mt + 1) * P], qt[:, dc, :],
                                start=(dc == 0), stop=(dc == DC - 1))
                        nc.scalar.activation(
                            out=expT[mt], in_=psc,
                            func=AF.Exp, bias=shift, scale=1.0)

                    fT = lntp.tile([P, DC, 512], BF16, tag="lnT")

                    def av_f(nb):
                        pf = ps.tile([P, 512], FP32, tag="ps")
                        for mt in range(MT):
                            nc.tensor.matmul(
                                pf, expT[mt][:, nb * P:(nb + 1) * P], v_sb[:, mt, :],
                                start=(mt == 0), stop=(mt == MT - 1))
                        # raw (unnormalized) f eviction -- the only op on the
                        # AV -> transpose critical path
                        f_sb = lnp.tile([P, D], BF16, tag="ln")
                        nc.scalar.activation(
                            out=f_sb, in_=pf, func=AF.Identity, bias=0.0, scale=1.0)
                        # LN stats (consumed only by the out-projection evict)
                        st = stat.tile([P, 6], FP32, tag="st")
                        nc.vector.bn_stats(out=st, in_=pf)
                        mv = stat.tile([P, 2], FP32, tag="mv")
                        nc.vector.bn_aggr(out=mv, in_=st)
                        # sqrt on Act is fine here: rstd is consumed only by
                        # the out-projection eviction (deferred LN), so the
                        # sqrt<->exp table swaps sit off the PE critical path.
                        sq = stat.tile([P, 1], FP32, tag="sq")
                        nc.scalar.activation(
                            out=sq, in_=mv[:, 1:2], func=AF.Sqrt,
                            bias=eps_t, scale=1.0)
                        rstd = stat.tile([P, 1], FP32, tag="rstd")
                        nc.vector.reciprocal(out=rstd, in_=sq)
                        negb = stat.tile([P, 1], FP32, tag="negb")
                        nc.vector.tensor_scalar(
                            out=negb, in0=mv[:, 0:1], scalar1=rstd, scalar2=mneg1,
                            op0=ALU.mult, op1=ALU.mult)
                        return f_sb, rstd, negb

                    def f_transpose(nb, f_sb):
                        for dc in range(DC):
                            pt = ps.tile([P, 512], BF16, tag="ps")
                            nc.tensor.transpose(
                                pt[:, :P], f_sb[:, dc * P:(dc + 1) * P], identb)
                            if dc < 2:
                                nc.vector.tensor_copy(
                                    out=fT[:, dc, nb * P:(nb + 1) * P], in_=pt[:, :P])
                            else:
                                nc.scalar.copy(
                                    out=fT[:, dc, nb * P:(nb + 1) * P], in_=pt[:, :P])

                    def out_chunk(nb, rstd, negb):
                        po = ps.tile([P, 512], FP32, tag="ps")
                        for dc in range(DC):
                            nc.tensor.matmul(
                                po[:, :F], fT[:, dc, nb * P:(nb + 1) * P],
                                wo_t[:, dc, :],
                                start=(dc == 0), stop=(dc == DC - 1))
                        # out = rstd*po + (-mu*rstd)*colsum(Wo') + bo'
                        o1 = outp.tile([P, F], FP32, tag="o1")
                        nc.scalar.activation(
                            out=o1, in_=po[:, :F], func=AF.Identity,
                            bias=0.0, scale=rstd)
                        o2 = outp.tile([P, F], FP32, tag="o2")
                        nc.scalar.activation(
                            out=o2, in_=cs_b, func=AF.Identity,
                            bias=0.0, scale=negb)
                        o3 = outp.tile([P, F], FP32, tag="o3")
                        nc.vector.tensor_tensor(out=o3, in0=o1, in1=o2, op=ALU.add)
                        o_sb = outp.tile([P, F], FP32, tag="o")
                        nc.vector.tensor_tensor(out=o_sb, in0=o3, in1=bo_b, op=ALU.add)
                        row0 = ns * 512 + nb * P
                        nc.gpsimd.dma_start(out=out[row0:row0 + P, :], in_=o_sb)

                    fs = [None] * NSUB
                    fs[0] = av_f(0)
                    fs[1] = av_f(1)
                    f_transpose(0, fs[0][0])
                    fs[2] = av_f(2)
                    out_chunk(0, fs[0][1], fs[0][2])
                    f_transpose(1, fs[1][0])
                    fs[3] = av_f(3)
                    out_chunk(1, fs[1][1], fs[1][2])
                    f_transpose(2, fs[2][0])
                    if ns + 1 < NS:
                        emit_qt(ns + 1)
                    out_chunk(2, fs[2][1], fs[2][2])
                    f_transpose(3, fs[3][0])
                    out_chunk(3, fs[3][1], fs[3][2])

            if n_loop == 1:
                body()
            elif loop_scope == "unroll":
                for _ in range(n_loop):
                    body()
            else:
                with tc.For_i(0, n_loop, 1):
                    body()

    nc.compile()
    return nc


_NC = {}


def _get_nc(n_loop=1, loop_scope="all"):
    key = (n_loop, loop_scope)
    if key not in _NC:
        _NC[key] = build(n_loop, loop_scope)
    return _NC[key]


def make_in_maps(inputs):
    x1 = np.ascontiguousarray(inputs["input1"], dtype=np.float32)
    x2 = np.ascontiguousarray(inputs["input2"], dtype=np.float32)
    gamma = np.asarray(inputs["ln_gamma"], dtype=np.float32)
    beta = np.asarray(inputs["ln_beta"], dtype=np.float32)
    wo = np.asarray(inputs["Wo"], dtype=np.float32)
    bo = np.asarray(inputs["bo"], dtype=np.float32)
    # Fold layernorm affine params into the output projection on the host:
    #   ln(x)*g + b then @Wo + bo  ==  ln_core(x) @ (g[:,None]*Wo) + (b@Wo + bo)
    wo_f = gamma[:, None] * wo                      # [D, F]
    bo_f = beta @ wo + bo                           # [F]

    def pack_x(x):
        # [2048, 512] -> [S, 128, KO, 512]: x[s*512+j, ko*128+kp] at [s, kp, ko, j]
        return np.ascontiguousarray(
            x.reshape(4, 512, KO, P).transpose(0, 3, 2, 1), dtype=np.float32)

    def pack_w(w):
        # [K, D] -> [128, KO, D]: w[ko*128+kp, d] at [kp, ko, d]
        return np.ascontiguousarray(
            w.reshape(KO, P, -1).transpose(1, 0, 2), dtype=np.float32)

    wop = np.ascontiguousarray(
        wo_f.reshape(DC, P, F).transpose(1, 0, 2)).astype(ml_dtypes.bfloat16)

    shared = {
        "wqp": pack_w(np.asarray(inputs["Wq"], dtype=np.float32)),
        "wkp": pack_w(np.asarray(inputs["Wk"], dtype=np.float32)),
        "wvp": pack_w(np.asarray(inputs["Wv"], dtype=np.float32)),
        "wop": wop,
        "bqp": np.ascontiguousarray(
            np.asarray(inputs["bq"], np.float32).reshape(DC, P).T),
        "bkp": np.ascontiguousarray(
            np.asarray(inputs["bk"], np.float32).reshape(DC, P).T),
        "bvb": np.ascontiguousarray(
            np.tile(np.asarray(inputs["bv"], np.float32)[None, :], (P, 1))),
        "bob": np.ascontiguousarray(np.tile(bo_f[None, :], (P, 1))),
        "csb": np.ascontiguousarray(
            np.tile(wo_f.sum(axis=0)[None, :].astype(np.float32), (P, 1))),
    }
    return [dict(shared, x1p=pack_x(x1[b]), x2p=pack_x(x2[b])) for b in range(B)]


def run(inputs, trace=False):
    nc = _get_nc(1)
    in_maps = make_in_maps(inputs)
    res = run_bass_kernel_spmd(nc, in_maps, list(range(B)), trace=trace)
    out = np.stack([res.results[b]["out"] for b in range(B)], axis=0)
    return out.astype(np.float32), res


def kernel(**inputs):
    out, _ = run(inputs, trace=False)
    return out


# revision 15
# speedup vs baseline: 1.0497x; 1.0497x over previous
"""Cross-attention Bass/Tile kernel for Trainium2, data-parallel over batch.

Problem (per batch element b, 8 of them, one per NeuronCore):
    q = x1 @ Wq + bq                      [2048, 512]
    k = x2 @ Wk + bk                      [2048, 512]
    v = x2 @ Wv + bv                      [2048, 512]
    scores  = q @ k.T                     [2048, 2048]  (no 1/sqrt(d))
    weights = softmax(scores, axis=-1)
    fused   = layer_norm(weights @ v) * gamma + beta
    out     = fused @ Wo + bo             [2048, 128]

Tricks relative to a naive lowering:
  * layer_norm is invariant to a per-row positive scale, so the softmax
    denominator is never computed: weights = exp(scores - 21) feed the AV
    matmul unnormalized and LN absorbs the scale.
  * x1/x2 are transposed AND tiled on the HOST (k-on-partitions layout), so
    the kernel has zero input transposes on the PE and zero PSUM->SBUF
    eviction copies for them.  Weights/biases are likewise host-packed;
    gamma/beta are folded into Wo/bo on the host.
  * rsqrt for LN is computed as exp(-0.5*ln(var+eps)): Ln/Exp/Identity all
    live in the same activation table set, so the Act engine never reloads
    tables (Sqrt would force 2 reloads x 1283ns per slice).
  * The post-LN path runs in bf16 (lnT transposes at 1.0 cycles/row, final
    projection as 128-wide natural-layout matmuls), so the final output
    needs no PE transposes back.  Everything through the AV matmul stays
    float32r (fp32 bits, 1 cycle/row at free-dim >= 256).
  * The prologue is chunk-pipelined: each 512-row chunk of x2 is consumed by
    K^T/V matmuls as soon as its DMA lands, instead of waiting for all of
    x2 to arrive.

build(n_loop=k) wraps the body in tc.For_i for slope benchmarking; the
graded kernel uses k=1.
"""

import numpy as np
import ml_dtypes

import concourse.bass as bass
import concourse.mybir as mybir
import concourse.tile as tile
from concourse import bacc
from concourse.bass_utils import run_bass_kernel_spmd
from concourse.masks import make_identity

# Problem dims (hardcoded per the harness contract)
B, N, M, K, D, F = 8, 2048, 2048, 512, 512, 128
P = 128
KO = K // P      # 4 contraction chunks over D_IN
DC = D // P      # 4 chunks over D_OUT
MT = M // P      # 16 m-chunks
MS = M // 512    # 4 m-slices of 512 (prologue chunks)
NS = N // 512    # 4 n-slices of 512
NSUB = 512 // P  # 4 n-subtiles per slice

FP32 = mybir.dt.float32
FP32R = mybir.dt.float32r
FP16 = mybir.dt.float16
BF16 = mybir.dt.bfloat16
EXP_SHIFT = -21.0
AF = mybir.ActivationFunctionType
ALU = mybir.AluOpType


def build(n_loop=1, loop_scope="all"):
    nc = bacc.Bacc(None, target_bir_lowering=False)

    x1p = nc.dram_tensor("x1p", [NS, P, KO, 512], FP32, kind="ExternalInput")
    x2p = nc.dram_tensor("x2p", [MS, P, KO, 512], FP32, kind="ExternalInput")
    wqp = nc.dram_tensor("wqp", [P, KO, D], FP32, kind="ExternalInput")
    wkp = nc.dram_tensor("wkp", [P, KO, D], FP32, kind="ExternalInput")
    wvp = nc.dram_tensor("wvp", [P, KO, D], FP32, kind="ExternalInput")
    wop = nc.dram_tensor("wop", [P, DC, F], BF16, kind="ExternalInput")
    bqp = nc.dram_tensor("bqp", [P, DC], FP32, kind="ExternalInput")
    bkp = nc.dram_tensor("bkp", [P, DC], FP32, kind="ExternalInput")
    bvb = nc.dram_tensor("bvb", [P, D], FP32, kind="ExternalInput")
    bob = nc.dram_tensor("bob", [P, F], FP32, kind="ExternalInput")
    csb = nc.dram_tensor("csb", [P, F], FP32, kind="ExternalInput")
    out = nc.dram_tensor("out", [N, F], FP32, kind="ExternalOutput")

    with tile.TileContext(nc) as tc:
        with (
            tc.tile_pool(name="const", bufs=1) as constp,
            tc.tile_pool(name="wtmp", bufs=1) as wtmp,
            tc.tile_pool(name="big", bufs=1) as bigp,
            tc.tile_pool(name="expp", bufs=MT) as expp,
            tc.tile_pool(name="x1tp", bufs=2) as x1tp,
            tc.tile_pool(name="qtp", bufs=2) as qtp,
            tc.tile_pool(name="lnp", bufs=3) as lnp,
            tc.tile_pool(name="lntp", bufs=2) as lntp,
            tc.tile_pool(name="outp", bufs=2) as outp,
            tc.tile_pool(name="stat", bufs=6) as stat,
            tc.tile_pool(name="ps", bufs=8, space="PSUM") as ps,
        ):
            # ---- constants (loaded once, outside any loop) ----
            ident = constp.tile([P, P], FP32)
            make_identity(nc, ident)
            identb = constp.tile([P, P], BF16)
            nc.vector.tensor_copy(out=identb, in_=ident)
            shift = constp.tile([P, 1], FP32)
            nc.vector.memset(shift, EXP_SHIFT)
            eps_t = constp.tile([P, 1], FP32)
            nc.vector.memset(eps_t, 1e-5)
            mhalf = constp.tile([P, 1], FP32)
            nc.vector.memset(mhalf, -0.5)
            mneg1 = constp.tile([P, 1], FP32)
            nc.vector.memset(mneg1, -1.0)
            bq_t = constp.tile([P, DC], FP32)
            nc.sync.dma_start(out=bq_t, in_=bqp[:, :])
            bk_t = constp.tile([P, DC], FP32)
            nc.sync.dma_start(out=bk_t, in_=bkp[:, :])
            bo_b = constp.tile([P, F], FP32)
            nc.sync.dma_start(out=bo_b, in_=bob[:, :])
            cs_b = constp.tile([P, F], FP32)
            nc.sync.dma_start(out=cs_b, in_=csb[:, :])
            bv_b = constp.tile([P, D], FP32)
            nc.sync.dma_start(out=bv_b, in_=bvb[:, :])
            wq_t = constp.tile([P, KO, D], FP32R)
            wo_t = constp.tile([P, DC, F], BF16)
            wk_t = wtmp.tile([P, KO, D], FP32R)
            wv_t = wtmp.tile([P, KO, D], FP32R)

            x2t = bigp.tile([P, KO, M], FP32R, tag="x2t", name="x2t")
            kt = bigp.tile([P, DC, M], FP32R, tag="kt", name="kt")
            v_sb = bigp.tile([P, MT, D], FP32R, tag="v", name="v_sb")
            x1ts = [x1tp.tile([P, KO, 512], FP32R, tag="x1t", name=f"x1t{i}")
                    for i in range(NS)]

            def body():
                # ---- input DMAs, ordered so the first consumers unblock
                # earliest: wk + x2 chunk0 feed the first K^T matmuls ----
                for ko in range(KO):
                    nc.gpsimd.dma_start(out=wk_t[:, ko, :], in_=wkp[:, ko, :])
                    nc.gpsimd.dma_start(out=x2t[:, ko, 0:512], in_=x2p[0][:, ko, :])
                for ko in range(KO):
                    nc.gpsimd.dma_start(out=wv_t[:, ko, :], in_=wvp[:, ko, :])
                    nc.gpsimd.dma_start(out=x2t[:, ko, 512:1024], in_=x2p[1][:, ko, :])
                nc.gpsimd.dma_start(out=wq_t, in_=wqp[:, :, :])
                nc.gpsimd.dma_start(out=x2t[:, :, 1024:1536], in_=x2p[2])
                nc.gpsimd.dma_start(out=x2t[:, :, 1536:2048], in_=x2p[3])
                nc.gpsimd.dma_start(out=wo_t, in_=wop[:, :, :])
                for i in range(2):
                    nc.gpsimd.dma_start(out=x1ts[i], in_=x1p[i])

                # ---- prologue: K^T and V, chunk-pipelined over x2 chunks ----
                for ms in range(MS):
                    mw = slice(ms * 512, (ms + 1) * 512)
                    for dc in range(DC):
                        pk = ps.tile([P, 512], FP32, tag="ps")
                        for ko in range(KO):
                            nc.tensor.matmul(
                                pk, wk_t[:, ko, dc * P:(dc + 1) * P],
                                x2t[:, ko, mw],
                                start=(ko == 0), stop=(ko == KO - 1))
                        nc.scalar.activation(
                            out=kt[:, dc, mw], in_=pk,
                            func=AF.Identity, bias=bk_t[:, dc:dc + 1], scale=1.0)
                    for mtl in range(4):
                        mt = ms * 4 + mtl
                        pv = ps.tile([P, 512], FP32, tag="ps")
                        for ko in range(KO):
                            nc.tensor.matmul(
                                pv, x2t[:, ko, mt * P:(mt + 1) * P], wv_t[:, ko, :],
                                start=(ko == 0), stop=(ko == KO - 1))
                        nc.vector.tensor_add(out=v_sb[:, mt, :], in0=pv, in1=bv_b)

                # ---- per 512-wide n-slice ----
                # Layer_norm is applied AFTER the final projection:
                #   ln(f) @ Wo' = rstd*(f@Wo') - (mu*rstd)*colsum(Wo'), with
                # rstd/mu per output row (n on partitions in the natural
                # layout), so the raw bf16 AV output is transposed directly and
                # the whole LN stats chain runs off the PE critical path.
                qts = [None] * NS

                def emit_qt(ns):
                    qt = qtp.tile([P, DC, 512], FP32R, tag="qt")
                    x1t = x1ts[ns]
                    for dc in range(DC):
                        pq = ps.tile([P, 512], FP32, tag="ps")
                        for ko in range(KO):
                            nc.tensor.matmul(
                                pq, wq_t[:, ko, dc * P:(dc + 1) * P], x1t[:, ko, :],
                                start=(ko == 0), stop=(ko == KO - 1))
                        nc.scalar.activation(
                            out=qt[:, dc, :], in_=pq,
                            func=AF.Identity, bias=bq_t[:, dc:dc + 1], scale=1.0)
                    qts[ns] = qt

                emit_qt(0)
                for ns in range(NS):
                    qt = qts[ns]

                    # transposed scores + fused exp eviction
                    expT = [expp.tile([P, 512], FP32R, tag="expT", name=f"expT{i}")
                            for i in range(MT)]
                    for mt in range(MT):
                        psc = ps.tile([P, 512], FP32, tag="ps")
                        for dc in range(DC):
                            nc.tensor.matmul(
                                psc, kt[:, dc, mt * P:(mt + 1) * P], qt[:, dc, :],
                                start=(dc == 0), stop=(dc == DC - 1))
                        nc.scalar.activation(
                            out=expT[mt], in_=psc,
                            func=AF.Exp, bias=shift, scale=1.0)

                    fT = lntp.tile([P, DC, 512], BF16, tag="lnT")

                    def av_f(nb):
                        pf = ps.tile([P, 512], FP32, tag="ps")
                        for mt in range(MT):
                            nc.tensor.matmul(
                                pf, expT[mt][:, nb * P:(nb + 1) * P], v_sb[:, mt, :],
                                start=(mt == 0), stop=(mt == MT - 1))
                        # raw (unnormalized) f eviction -- the only op on the
                        # AV -> transpose critical path
                        f_sb = lnp.tile([P, D], BF16, tag="ln")
                        nc.scalar.activation(
                            out=f_sb, in_=pf, func=AF.Identity, bias=0.0, scale=1.0)
                        # LN stats (consumed only by the out-projection evict)
                        st = stat.tile([P, 6], FP32, tag="st")
                        nc.vector.bn_stats(out=st, in_=pf)
                        mv = stat.tile([P, 2], FP32, tag="mv")
                        nc.vector.bn_aggr(out=mv, in_=st)
                        # sqrt on Act is fine here: rstd is consumed only by
                        # the out-projection eviction (deferred LN), so the
                        # sqrt<->exp table swaps sit off the PE critical path.
                        sq = stat.tile([P, 1], FP32, tag="sq")
                        nc.scalar.activation(
                            out=sq, in_=mv[:, 1:2], func=AF.Sqrt,
                            bias=eps_t, scale=1.0)
                        rstd = stat.tile([P, 1], FP32, tag="rstd")
                        nc.vector.reciprocal(out=rstd, in_=sq)
                        negb = stat.tile([P, 1], FP32, tag="negb")
                        nc.vector.tensor_scalar(
                            out=negb, in0=mv[:, 0:1], scalar1=rstd, scalar2=mneg1,
                            op0=ALU.mult, op1=ALU.mult)
                        return f_sb, rstd, negb

                    def f_transpose(nb, f_sb):
                        for dc in range(DC):
                            pt = ps.tile([P, 512], BF16, tag="ps")
                            nc.tensor.transpose(
                                pt[:, :P], f_sb[:, dc * P:(dc + 1) * P], identb)
                            if dc < 2:
                                nc.vector.tensor_copy(
                                    out=fT[:, dc, nb * P:(nb + 1) * P], in_=pt[:, :P])
                            else:
                                nc.scalar.copy(
                                    out=fT[:, dc, nb * P:(nb + 1) * P], in_=pt[:, :P])

                    def out_chunk(nb, rstd, negb):
                        po = ps.tile([P, 512], FP32, tag="ps")
                        for dc in range(DC):
                            nc.tensor.matmul(
                                po[:, :F], fT[:, dc, nb * P:(nb + 1) * P],
                                wo_t[:, dc, :],
                                start=(dc == 0), stop=(dc == DC - 1))
                        # out = rstd*po + (-mu*rstd)*colsum(Wo') + bo'
                        o1 = outp.tile([P, F], FP32, tag="o1")
                        nc.scalar.activation(
                            out=o1, in_=po[:, :F], func=AF.Identity,
                            bias=0.0, scale=rstd)
                        o2 = outp.tile([P, F], FP32, tag="o2")
                        nc.scalar.activation(
                            out=o2, in_=cs_b, func=AF.Identity,
                            bias=0.0, scale=negb)
                        o3 = outp.tile([P, F], FP32, tag="o3")
                        nc.vector.tensor_tensor(out=o3, in0=o1, in1=o2, op=ALU.add)
                        o_sb = outp.tile([P, F], FP32, tag="o")
                        nc.vector.tensor_tensor(out=o_sb, in0=o3, in1=bo_b, op=ALU.add)
                        row0 = ns * 512 + nb * P
                        nc.gpsimd.dma_start(out=out[row0:row0 + P, :], in_=o_sb)

                    fs = [None] * NSUB
                    fs[0] = av_f(0)
                    fs[1] = av_f(1)
                    f_transpose(0, fs[0][0])
                    fs[2] = av_f(2)
                    out_chunk(0, fs[0][1], fs[0][2])
                    f_transpose(1, fs[1][0])
                    fs[3] = av_f(3)
                    out_chunk(1, fs[1][1], fs[1][2])
                    if ns + 2 < NS:
                        nc.gpsimd.dma_start(out=x1ts[ns + 2], in_=x1p[ns + 2])
                    f_transpose(2, fs[2][0])
                    if ns + 1 < NS:
                        emit_qt(ns + 1)
                    out_chunk(2, fs[2][1], fs[2][2])
                    f_transpose(3, fs[3][0])
                    out_chunk(3, fs[3][1], fs[3][2])

            if n_loop == 1:
                body()
            elif loop_scope == "unroll":
                for _ in range(n_loop):
                    body()
            else:
                with tc.For_i(0, n_loop, 1):
                    body()

    nc.compile()
    return nc


_NC = {}


def _get_nc(n_loop=1, loop_scope="all"):
    key = (n_loop, loop_scope)
    if key not in _NC:
        _NC[key] = build(n_loop, loop_scope)
    return _NC[key]


def make_in_maps(inputs):
    x1 = np.ascontiguousarray(inputs["input1"], dtype=np.float32)
    x2 = np.ascontiguousarray(inputs["input2"], dtype=np.float32)
    gamma = np.asarray(inputs["ln_gamma"], dtype=np.float32)
    beta = np.asarray(inputs["ln_beta"], dtype=np.float32)
    wo = np.asarray(inputs["Wo"], dtype=np.float32)
    bo = np.asarray(inputs["bo"], dtype=np.float32)
    # Fold layernorm affine params into the output projection on the host:
    #   ln(x)*g + b then @Wo + bo  ==  ln_core(x) @ (g[:,None]*Wo) + (b@Wo + bo)
    wo_f = gamma[:, None] * wo                      # [D, F]
    bo_f = beta @ wo + bo                           # [F]

    def pack_x(x):
        # [2048, 512] -> [S, 128, KO, 512]: x[s*512+j, ko*128+kp] at [s, kp, ko, j]
        return np.ascontiguousarray(
            x.reshape(4, 512, KO, P).transpose(0, 3, 2, 1), dtype=np.float32)

    def pack_w(w):
        # [K, D] -> [128, KO, D]: w[ko*128+kp, d] at [kp, ko, d]
        return np.ascontiguousarray(
            w.reshape(KO, P, -1).transpose(1, 0, 2), dtype=np.float32)

    wop = np.ascontiguousarray(
        wo_f.reshape(DC, P, F).transpose(1, 0, 2)).astype(ml_dtypes.bfloat16)

    shared = {
        "wqp": pack_w(np.asarray(inputs["Wq"], dtype=np.float32)),
        "wkp": pack_w(np.asarray(inputs["Wk"], dtype=np.float32)),
        "wvp": pack_w(np.asarray(inputs["Wv"], dtype=np.float32)),
        "wop": wop,
        "bqp": np.ascontiguousarray(
            np.asarray(inputs["bq"], np.float32).reshape(DC, P).T),
        "bkp": np.ascontiguousarray(
            np.asarray(inputs["bk"], np.float32).reshape(DC, P).T),
        "bvb": np.ascontiguousarray(
            np.tile(np.asarray(inputs["bv"], np.float32)[None, :], (P, 1))),
        "bob": np.ascontiguousarray(np.tile(bo_f[None, :], (P, 1))),
        "csb": np.ascontiguousarray(
            np.tile(wo_f.sum(axis=0)[None, :].astype(np.float32), (P, 1))),
    }
    return [dict(shared, x1p=pack_x(x1[b]), x2p=pack_x(x2[b])) for b in range(B)]


def run(inputs, trace=False):
    nc = _get_nc(1)
    in_maps = make_in_maps(inputs)
    res = run_bass_kernel_spmd(nc, in_maps, list(range(B)), trace=trace)
    out = np.stack([res.results[b]["out"] for b in range(B)], axis=0)
    return out.astype(np.float32), res


def kernel(**inputs):
    out, _ = run(inputs, trace=False)
    return out
